# revision 1
# baseline (speedup 1.0000x reference)
"""GQA attention block on 8 Trainium2 cores.

Sharding: data-parallel over batch B=2 x tensor-parallel over the 4 KV groups
(cores 0-3 -> batch 0 groups 0-3, cores 4-7 -> batch 1 groups 0-3).
Each core computes Q/K/V projections for its group, attention for its 4 query
heads, and a row-sharded partial of the output projection.  The host sums the
4 partials per batch and adds the output bias.

All matmuls run in float32r (full-rate fp32 mode on the PE array);
accumulation is fp32 in PSUM.

On-device layout trick: the host feeds x pre-transposed (xT = x[b].T), so
every projection matmul can consume it directly as the moving operand with
the contraction dim (d_model) on partitions -- no on-device transposes except
16 small 128x128 PE transposes to turn V^T into V.
"""
import sys

sys.path.insert(0, "/opt/trn_rl_repo")

import math
from contextlib import ExitStack

import numpy as np

import concourse.bacc as bacc
import concourse.tile as tile
import concourse.mybir as mybir
from concourse.bass_utils import run_bass_kernel_spmd
from concourse.masks import make_identity

F32 = mybir.dt.float32
F32R = mybir.dt.float32r
AF = mybir.ActivationFunctionType

D = 2048          # d_model
S = 2048          # sequence length
HD = 128          # head dim
R = 4             # q heads per kv group (on one core)
GD = R * HD       # 512: q-projection width per core
KT_TILES = S // 128   # 16 key-time tiles
KD_TILES = D // 128   # 16 contraction tiles for projections
N_SC = 4          # s-chunks of 512
SC = S // N_SC    # 512
SCALE = 1.0 / math.sqrt(HD)

_CACHED = {}


def _build():
    nc = bacc.Bacc("TRN2", target_bir_lowering=False, debug=False, num_devices=8)

    XT = nc.dram_tensor("xt", [D, S], F32R, kind="ExternalInput")
    WQ = nc.dram_tensor("wq", [D, GD], F32R, kind="ExternalInput")
    WK = nc.dram_tensor("wk", [D, HD], F32R, kind="ExternalInput")
    WV = nc.dram_tensor("wv", [D, HD], F32R, kind="ExternalInput")
    WO = nc.dram_tensor("wo", [GD, D], F32R, kind="ExternalInput")
    BQ = nc.dram_tensor("bq", [128, R], F32, kind="ExternalInput")
    BK = nc.dram_tensor("bk", [128, 1], F32, kind="ExternalInput")
    BV = nc.dram_tensor("bv", [128, 1], F32, kind="ExternalInput")
    OUT = nc.dram_tensor("out", [S, D], F32, kind="ExternalOutput")

    with tile.TileContext(nc) as tc, ExitStack() as ctx:
        # ---- long-lived tiles ----
        lp = ctx.enter_context(tc.tile_pool(name="long", bufs=1))
        qt_sb = lp.tile([128, R, S], F32R)        # Q^T per head: [dq, h, s]
        kt_sb = lp.tile([128, S], F32R)           # K^T: [dk, t]
        vt_sb = lp.tile([128, S], F32R)           # V^T: [dv, t]
        v_sb = lp.tile([128, KT_TILES, HD], F32R) # V natural: [t_sub, t_tile, dv]
        bq_sb = lp.tile([128, R], F32)
        bk_sb = lp.tile([128, 1], F32)
        bv_sb = lp.tile([128, 1], F32)
        ones_col = lp.tile([128, 1], F32R)
        ones_row = lp.tile([1, 128], F32R)
        ident = lp.tile([128, 128], F32R)

        nc.sync.dma_start(bq_sb[:], BQ.ap())
        nc.sync.dma_start(bk_sb[:], BK.ap())
        nc.sync.dma_start(bv_sb[:], BV.ap())

        tmp_f = lp.tile([128, 128], F32)
        nc.gpsimd.memset(tmp_f[:], 1.0)
        nc.vector.tensor_copy(ones_col[:], tmp_f[:, 0:1])
        nc.vector.tensor_copy(ones_row[:], tmp_f[0:1, 0:128])
        make_identity(nc, tmp_f[:])
        nc.vector.tensor_copy(ident[:], tmp_f[:])

        # ---- phase A: projections ----
        with ExitStack() as actx:
            wp = actx.enter_context(tc.tile_pool(name="wqkv", bufs=1))
            xp = actx.enter_context(tc.tile_pool(name="xt", bufs=2))
            psa = actx.enter_context(tc.tile_pool(name="psa", bufs=4, space="PSUM"))

            wq_sb = wp.tile([128, KD_TILES, GD], F32R)
            wk_sb = wp.tile([128, KD_TILES, HD], F32R)
            wv_sb = wp.tile([128, KD_TILES, HD], F32R)
            wq_r = WQ.ap().rearrange("(ko p) n -> p ko n", p=128)
            wk_r = WK.ap().rearrange("(ko p) n -> p ko n", p=128)
            wv_r = WV.ap().rearrange("(ko p) n -> p ko n", p=128)

            for sc in range(N_SC):
                xt = xp.tile([128, KD_TILES, SC], F32R, tag="xt")
                xt_r = XT.ap()[:, sc * SC:(sc + 1) * SC].rearrange(
                    "(ko p) s -> p ko s", p=128
                )
                # interleave per-k so the k=0 matmuls can start almost
                # immediately (weights ride along with the first chunk)
                for k in range(KD_TILES):
                    nc.sync.dma_start(xt[:, k, :], xt_r[:, k, :])
                    if sc == 0:
                        nc.sync.dma_start(wq_sb[:, k, :], wq_r[:, k, :])
                        nc.sync.dma_start(wk_sb[:, k, :], wk_r[:, k, :])
                        nc.sync.dma_start(wv_sb[:, k, :], wv_r[:, k, :])
                # Q^T for the 4 heads
                for dq in range(R):
                    ps = psa.tile([128, SC], F32, tag="psa")
                    for k in range(KD_TILES):
                        nc.tensor.matmul(
                            ps[:],
                            lhsT=wq_sb[:, k, dq * 128:(dq + 1) * 128],
                            rhs=xt[:, k, :],
                            start=(k == 0),
                            stop=(k == KD_TILES - 1),
                        )
                    nc.scalar.activation(
                        qt_sb[:, dq, sc * SC:(sc + 1) * SC], ps[:],
                        AF.Identity, bias=bq_sb[:, dq:dq + 1],
                    )
                # K^T
                ps = psa.tile([128, SC], F32, tag="psa")
                for k in range(KD_TILES):
                    nc.tensor.matmul(
                        ps[:], lhsT=wk_sb[:, k, :], rhs=xt[:, k, :],
                        start=(k == 0), stop=(k == KD_TILES - 1),
                    )
                nc.scalar.activation(
                    kt_sb[:, sc * SC:(sc + 1) * SC], ps[:],
                    AF.Identity, bias=bk_sb[:],
                )
                # V^T
                ps = psa.tile([128, SC], F32, tag="psa")
                for k in range(KD_TILES):
                    nc.tensor.matmul(
                        ps[:], lhsT=wv_sb[:, k, :], rhs=xt[:, k, :],
                        start=(k == 0), stop=(k == KD_TILES - 1),
                    )
                nc.scalar.activation(
                    vt_sb[:, sc * SC:(sc + 1) * SC], ps[:],
                    AF.Identity, bias=bv_sb[:],
                )

            # V^T -> V natural (16 PE transposes)
            pst = actx.enter_context(tc.tile_pool(name="pst", bufs=2, space="PSUM"))
            for t in range(KT_TILES):
                pt_ps = pst.tile([128, 128], F32R, tag="pst")
                nc.tensor.transpose(
                    pt_ps[:], vt_sb[:, t * 128:(t + 1) * 128], ident[:]
                )
                nc.vector.tensor_copy(v_sb[:, t, :], pt_ps[:])

        # ---- phase B: attention + out-proj ----
        with ExitStack() as bctx:
            wop = bctx.enter_context(tc.tile_pool(name="wo", bufs=1))
            wo_sb = wop.tile([128, R, D], F32R)
            nc.sync.dma_start(wo_sb[:], WO.ap().rearrange("(h p) n -> p h n", p=128))

            pss = bctx.enter_context(tc.tile_pool(name="pss", bufs=2, space="PSUM"))
            pso = bctx.enter_context(tc.tile_pool(name="pso", bufs=2, space="PSUM"))
            psm = bctx.enter_context(tc.tile_pool(name="psm", bufs=2, space="PSUM"))
            ptp = bctx.enter_context(tc.tile_pool(name="ptp", bufs=4))
            accp = bctx.enter_context(tc.tile_pool(name="accp", bufs=3))
            otp = bctx.enter_context(tc.tile_pool(name="otp", bufs=3))
            outp = bctx.enter_context(tc.tile_pool(name="outp", bufs=4))

            def t_loop(sc, h, ot_sb, mid_cb=None):
                """scores -> exp -> attn accumulation + partial denom sums.
                Returns state consumed later by tail().  mid_cb is emitted
                after group 2 (pipelines the previous head's tail here)."""
                ps_o = pso.tile([128, SC], F32, tag="pso", name="ps_o")
                parts = [accp.tile([128, SC], F32, tag=f"acc{j}", name=f"acc{j}")
                         for j in range(4)]
                for tg in range(KT_TILES // 2):
                    if tg == 2 and mid_cb is not None:
                        mid_cb()
                    ps_s = pss.tile([128, 2, SC], F32, tag="pss", name="ps_s")
                    for i in range(2):
                        t = tg * 2 + i
                        nc.tensor.matmul(
                            ps_s[:, i, :],
                            lhsT=kt_sb[:, t * 128:(t + 1) * 128],
                            rhs=qt_sb[:, h, sc * SC:(sc + 1) * SC],
                            start=True, stop=True,
                        )
                    pt = ptp.tile([128, 2, SC], F32R, tag="pt", name="pt")
                    nc.scalar.activation(pt[:], ps_s[:], AF.Exp, scale=SCALE)
                    for i in range(2):
                        t = tg * 2 + i
                        nc.tensor.matmul(
                            ps_o[:],
                            lhsT=v_sb[:, t, :],
                            rhs=pt[:, i, :],
                            start=(t == 0),
                            stop=(t == KT_TILES - 1),
                            skip_group_check=True,
                        )
                    # partial denominator sums: parts[i + 2*(tg>=4)]
                    for i in range(2):
                        j = i + 2 * (tg >= 4)
                        src = pt[:, i, :].bitcast(F32)
                        if tg in (0, 4):
                            nc.vector.tensor_copy(parts[j][:], src)
                        else:
                            nc.vector.tensor_add(parts[j][:], parts[j][:], src)
                    if tg == 3:  # early merge of first half (off critical path)
                        nc.vector.tensor_add(parts[0][:], parts[0][:], parts[1][:])
                return ps_o, parts

            def tail(sc, h, ot_sb, ps_o, parts):
                """denominator -> reciprocal -> broadcast -> normalize.
                ps_d/ps_b live inside one pss-tagged slot (bank sharing)."""
                acc_r = accp.tile([128, SC], F32R, tag="acc_r", name="acc_r")
                nc.vector.tensor_add(parts[2][:], parts[2][:], parts[3][:])
                nc.vector.tensor_add(acc_r[:], parts[0][:], parts[2][:])
                ps_d = psm.tile([1, SC], F32, tag="psm", name="ps_d")
                nc.tensor.matmul(
                    ps_d[:], lhsT=ones_col[:], rhs=acc_r[:], start=True, stop=True
                )
                recip = accp.tile([1, SC], F32, tag="recip", name="recip")
                nc.vector.reciprocal_approx_fast(recip[:], ps_d[:])
                recip_r = accp.tile([1, SC], F32R, tag="recip_r", name="recip_r")
                nc.vector.tensor_copy(recip_r[:], recip[:])
                ps_b = psm.tile([128, SC], F32, tag="psm", name="ps_b")
                nc.tensor.matmul(
                    ps_b[:], lhsT=ones_row[:], rhs=recip_r[:],
                    start=True, stop=True,
                )
                bc = accp.tile([128, SC], F32, tag="bc", name="bc")
                nc.scalar.copy(bc[:], ps_b[:])
                nc.vector.tensor_mul(ot_sb[:, h, :], ps_o[:], bc[:])

            def out_proj(sc, ot_sb):
                for st in range(SC // 128):
                    for oc in range(D // 512):
                        ps_f = psm.tile([128, 512], F32, tag="psm", name="ps_f")
                        for dv in range(R):
                            nc.tensor.matmul(
                                ps_f[:],
                                lhsT=ot_sb[:, dv, st * 128:(st + 1) * 128],
                                rhs=wo_sb[:, dv, oc * 512:(oc + 1) * 512],
                                start=(dv == 0),
                                stop=(dv == R - 1),
                            )
                        o_t = outp.tile([128, 512], F32, tag="out", name="o_t")
                        if (st + oc) % 2 == 0:
                            nc.scalar.copy(o_t[:], ps_f[:])
                        else:
                            nc.vector.tensor_copy(o_t[:], ps_f[:])
                        nc.sync.dma_start(
                            OUT.ap()[
                                sc * SC + st * 128: sc * SC + (st + 1) * 128,
                                oc * 512:(oc + 1) * 512,
                            ],
                            o_t[:],
                        )

            # software pipeline: tail(i-1) is emitted after t_loop(i) so the
            # PE never sits in-order behind the DVE denominator chain; the
            # out-proj of chunk sc is emitted after t_loop(sc+1, h=0).
            for sc in range(N_SC):
                ot_sb = otp.tile([128, R, SC], F32R, tag="ot", name="ot_sb")
                for h in range(R):
                    ps_o, parts = t_loop(sc, h, ot_sb)
                    tail(sc, h, ot_sb, ps_o, parts)
                out_proj(sc, ot_sb)

    nc.compile()
    return nc


def _get_nc():
    if "nc" not in _CACHED:
        _CACHED["nc"] = _build()
    return _CACHED["nc"]


def _make_in_maps(x, Wq, bq, Wk, bk, Wv, bv, Wo):
    in_maps = []
    xts = [np.ascontiguousarray(x[b].T) for b in range(2)]
    for core in range(8):
        b, g = divmod(core, 4)
        in_maps.append({
            "xt": xts[b],
            "wq": np.ascontiguousarray(Wq[:, g * GD:(g + 1) * GD]),
            "wk": np.ascontiguousarray(Wk[:, g * HD:(g + 1) * HD]),
            "wv": np.ascontiguousarray(Wv[:, g * HD:(g + 1) * HD]),
            "wo": np.ascontiguousarray(Wo[g * GD:(g + 1) * GD, :]),
            "bq": np.ascontiguousarray(
                bq[g * GD:(g + 1) * GD].reshape(R, 128).T
            ),
            "bk": bk[g * HD:(g + 1) * HD].reshape(HD, 1).copy(),
            "bv": bv[g * HD:(g + 1) * HD].reshape(HD, 1).copy(),
        })
    return in_maps


def kernel(x, Wq, bq, Wk, bk, Wv, bv, Wo, bo, _trace=False):
    x = np.asarray(x, dtype=np.float32)
    nc = _get_nc()
    in_maps = _make_in_maps(
        x,
        np.asarray(Wq, np.float32), np.asarray(bq, np.float32),
        np.asarray(Wk, np.float32), np.asarray(bk, np.float32),
        np.asarray(Wv, np.float32), np.asarray(bv, np.float32),
        np.asarray(Wo, np.float32),
    )
    res = run_bass_kernel_spmd(nc, in_maps, list(range(8)), trace=_trace)
    bo = np.asarray(bo, np.float32)
    out = np.empty((2, S, D), np.float32)
    for b in range(2):
        acc = res.results[b * 4]["out"].astype(np.float32)
        for g in range(1, 4):
            acc = acc + res.results[b * 4 + g]["out"]
        out[b] = acc + bo[None, :]
    if _trace:
        return out, res
    return out

